# revision 3
# baseline (speedup 1.0000x reference)
"""AgentMatchingDecoder Trainium2 kernel — 8-core SPMD, query(j)-sharded.

Math: with B=2, softmax over the batch axis collapses to a sigmoid:
  p0 = sigmoid(d), p1 = 1-p0,  d[i,j] = l0[i,j]-l1[i,j]
  d = sum_a' U[a',i] V[a',j]   (K=512 extended contraction)
  U = [AS0; AS1; onehotA0; onehotA1]      ([a,i] layouts)
  V = [QA0^T; -QA1^T; 1e6*ohQ0^T; -1e6*ohQ1^T]
dec1 uses the identity sum_i (1-p0) vs1 = colsum(vs1) - sum_i p0 vs1,
with colsum(vs1) computed on the host from the raw inputs.
Each core owns j in [512k, 512k+512) -> conv-input channels [32k,32k+32);
an AllToAll exchanges channel blocks for spatial halo rows so core k
computes output rows [8k, 8k+8).
"""
import os
import sys

import numpy as np

sys.path.insert(0, "/opt/trn_rl_repo")

B, NA, HW, C = 2, 128, 4096, 256
DFF, H, W = 2048, 64, 64
NCORES = 8
JS = HW // NCORES           # 512
RS = H // NCORES            # 8
SCALE = 1.0 / np.sqrt(C // 8)
BIG = 1.0e6

_CACHE = {}


def _build_nc():
    import concourse.bass as bass
    import concourse.mybir as mybir
    from concourse import bacc, tile
    from concourse.masks import make_identity

    f32 = mybir.dt.float32
    u32 = mybir.dt.uint32
    i32 = mybir.dt.int32
    AF = mybir.ActivationFunctionType
    OP = mybir.AluOpType

    nc = bacc.Bacc("TRN2", target_bir_lowering=False, debug=False,
                   num_devices=NCORES)

    def inp(name, shape):
        return nc.dram_tensor(name, list(shape), f32, kind="ExternalInput").ap()

    supp_t = inp("supp_t", [B, 2, 128, HW])
    query_t = inp("query_t", [B, 2, 128, JS])
    tok_t = inp("tok_t", [B, 2, 128, NA])
    wq_d = inp("wq", [128, 512])
    wk_d = inp("wk", [128, 512])
    wks_d = inp("wks", [128, 512])
    wvs_d = inp("wvs", [128, 512])
    w1_d = inp("w1", [2, 128, DFF])
    w2_d = inp("w2", [16, 128, C])
    w3_d = inp("w3", [128, 576])
    w1c_d = inp("w1c", [32, 27])
    bqa_s_d = inp("bqa_s", [128, 2])    # scaled bqa (for qa^T)
    bka_s_d = inp("bka_s", [128, 2])    # scaled bka
    bqa_c_d = inp("bqa_c", [128, 2])    # unscaled bqa (for qq^T)
    bks_c_d = inp("bks_c", [128, 2])
    bvsb_d = inp("bvs_bcast", [128, C])
    b1_d = inp("b1c", [128, 16])
    b2_d = inp("b2c", [128, 2])
    csv1_d = inp("cs_vs1", [128, 2])
    rowmask_d = inp("rowmask", [32, 640])

    out_ext = nc.dram_tensor("out", [B, 3, RS, W], f32, kind="ExternalOutput").ap()

    # internal DRAM
    x_dram = nc.dram_tensor("x_scratch", [B, 32 * H * W], f32).ap()
    a2a_in = nc.dram_tensor("a2a_in", [NCORES, B, 32, 12, W], f32).ap()
    a2a_out = nc.dram_tensor("a2a_out", [NCORES, B, 32, 12, W], f32).ap()

    with tile.TileContext(nc) as tc:
        with (
            tc.tile_pool(name="const", bufs=1) as cpool,
            tc.tile_pool(name="pers", bufs=1) as pers,
            tc.tile_pool(name="stream", bufs=3) as spool,
            tc.tile_pool(name="psw", bufs=4, space="PSUM") as ppw,
            tc.tile_pool(name="psdec", bufs=4, space="PSUM") as ppd,
        ):
            def ctile(shape, name, dtype=f32):
                return cpool.tile(shape, dtype, name=name, tag=name)

            def ptile(shape, name, dtype=f32):
                return pers.tile(shape, dtype, name=name, tag=name)

            def stile(shape, tag, dtype=f32, bufs=3):
                return spool.tile(shape, dtype, name=tag, tag=tag, bufs=bufs)

            def wps(shape=(128, 512)):
                return ppw.tile(list(shape), f32, name="w", tag="w", bufs=4)

            dma = nc.sync.dma_start
            mm = nc.tensor.matmul

            # ---------------- constants ----------------
            ident = ctile([128, 128], "ident")
            make_identity(nc, ident)
            iota_i = ctile([128, 128], "iota_i", i32)
            nc.gpsimd.iota(iota_i, pattern=[[1, 128]], base=0, channel_multiplier=0)
            iota_f = ctile([128, 128], "iota_f")
            nc.vector.tensor_copy(iota_f, iota_i)

            wq_sb = ctile([128, 512], "wq_sb"); dma(wq_sb, wq_d)
            wk_sb = ctile([128, 512], "wk_sb"); dma(wk_sb, wk_d)
            wks_sb = ctile([128, 512], "wks_sb"); dma(wks_sb, wks_d)
            wvs_sb = ctile([128, 512], "wvs_sb"); dma(wvs_sb, wvs_d)
            w3_sb = ctile([128, 576], "w3_sb"); dma(w3_sb, w3_d)
            w1c_sb = ctile([32, 27], "w1c_sb"); dma(w1c_sb, w1c_d)
            bqa_s = ctile([128, 2], "bqa_s_sb"); dma(bqa_s, bqa_s_d)
            bka_s = ctile([128, 2], "bka_s_sb"); dma(bka_s, bka_s_d)
            bqa_c = ctile([128, 2], "bqa_c_sb"); dma(bqa_c, bqa_c_d)
            bks_c = ctile([128, 2], "bks_c_sb"); dma(bks_c, bks_c_d)
            bvsb = ctile([128, C], "bvsb_sb"); dma(bvsb, bvsb_d)
            b1_sb = ctile([128, 16], "b1_sb"); dma(b1_sb, b1_d)
            b2_sb = ctile([128, 2], "b2_sb"); dma(b2_sb, b2_d)
            csv1 = ctile([128, 2], "csv1_sb"); dma(csv1, csv1_d)
            rowmask = ctile([32, 640], "rowmask_sb"); dma(rowmask, rowmask_d)
            zeros_sb = ctile([32, 128], "zeros_sb")
            nc.gpsimd.memset(zeros_sb, 0.0)

            tok_sb = ctile([128, 4 * NA], "tok_sb")
            for b in range(B):
                for cc in range(2):
                    dma(tok_sb[:, (b * 2 + cc) * NA:(b * 2 + cc + 1) * NA],
                        tok_t[b, cc])
            qt_sb = ctile([128, 4 * JS], "qt_sb")
            for b in range(B):
                for cc in range(2):
                    dma(qt_sb[:, (b * 2 + cc) * JS:(b * 2 + cc + 1) * JS],
                        query_t[b, cc])

            # ---------------- phase 1: qa^T, ka^T, qq^T ----------------
            qaT = ptile([128, 4 * NA], "qaT")      # (b, mo) chunks, scaled
            kaT = ptile([128, 4 * NA], "kaT")
            qqT = ptile([128, 4 * JS], "qqT")      # (b, cc') chunks
            for b in range(B):
                for mo in range(2):
                    ps = wps((128, NA))
                    for cc in range(2):
                        mm(ps, wq_sb[:, cc * 256 + mo * 128: cc * 256 + mo * 128 + 128],
                           tok_sb[:, (b * 2 + cc) * NA:(b * 2 + cc + 1) * NA],
                           start=(cc == 0), stop=(cc == 1))
                    nc.vector.tensor_scalar(
                        qaT[:, (b * 2 + mo) * NA:(b * 2 + mo + 1) * NA], ps,
                        scalar1=SCALE, scalar2=bqa_s[:, mo:mo + 1],
                        op0=OP.mult, op1=OP.add)
                    ps2 = wps((128, NA))
                    for cc in range(2):
                        mm(ps2, wk_sb[:, cc * 256 + mo * 128: cc * 256 + mo * 128 + 128],
                           tok_sb[:, (b * 2 + cc) * NA:(b * 2 + cc + 1) * NA],
                           start=(cc == 0), stop=(cc == 1))
                    nc.vector.tensor_scalar(
                        kaT[:, (b * 2 + mo) * NA:(b * 2 + mo + 1) * NA], ps2,
                        scalar1=SCALE, scalar2=bka_s[:, mo:mo + 1],
                        op0=OP.mult, op1=OP.add)
                    ps3 = wps()
                    for cc in range(2):
                        mm(ps3, wq_sb[:, cc * 256 + mo * 128: cc * 256 + mo * 128 + 128],
                           qt_sb[:, (b * 2 + cc) * JS:(b * 2 + cc + 1) * JS],
                           start=(cc == 0), stop=(cc == 1))
                    nc.vector.tensor_scalar(
                        qqT[:, (b * 2 + mo) * JS:(b * 2 + mo + 1) * JS], ps3,
                        scalar1=bqa_c[:, mo:mo + 1], scalar2=None, op0=OP.add)

            # ---------------- phase 2: QA^T (V rows), Q argmax, ohQ ----------------
            QAT = ptile([128, 2 * JS], "QAT")      # [a, j] per b (b1 negated)
            Vq = ptile([128, 2 * JS], "Vq")        # [a, j] per b: +-1e6 one-hot(Q)
            for b in range(B):
                ps = wps()
                for cc in range(2):
                    mm(ps, kaT[:, (b * 2 + cc) * NA:(b * 2 + cc + 1) * NA],
                       qqT[:, (b * 2 + cc) * JS:(b * 2 + cc + 1) * JS],
                       start=(cc == 0), stop=(cc == 1))
                if b == 0:
                    nc.vector.tensor_copy(QAT[:, :JS], ps)
                else:
                    nc.vector.tensor_scalar(QAT[:, JS:], ps, scalar1=-1.0,
                                            scalar2=None, op0=OP.mult)
                for jt in range(4):
                    psq = wps((128, NA))
                    for cc in range(2):
                        mm(psq, qqT[:, (b * 2 + cc) * JS + jt * 128:
                                    (b * 2 + cc) * JS + jt * 128 + 128],
                           kaT[:, (b * 2 + cc) * NA:(b * 2 + cc + 1) * NA],
                           start=(cc == 0), stop=(cc == 1))
                    qat_sb = stile([128, NA], "qat_sb")
                    nc.vector.tensor_copy(qat_sb, psq)
                    mx = stile([128, 8], "mxq")
                    nc.vector.max(mx, qat_sb)
                    idx = stile([128, 8], "idxq", dtype=u32)
                    nc.vector.max_index(idx, mx, qat_sb)
                    qcol = stile([128, 1], "qcol")
                    nc.vector.tensor_copy(qcol, idx[:, 0:1])
                    ohqt = stile([128, 128], "ohqt")
                    nc.vector.tensor_scalar(ohqt, iota_f, scalar1=qcol,
                                            scalar2=(BIG if b == 0 else -BIG),
                                            op0=OP.is_equal, op1=OP.mult)
                    pst = wps((128, 128))
                    nc.tensor.transpose(pst, ohqt, ident)
                    nc.vector.tensor_copy(
                        Vq[:, b * JS + jt * 128: b * JS + jt * 128 + 128], pst)

            # ---------------- main fused loop over support tiles ----------------
            dec_ps = [ppd.tile([128, JS], f32, name=f"dec{q}", tag="dec", bufs=4)
                      for q in range(4)]           # (b, mo)
            for ic8 in range(8):
                as_tiles = {}
                vs_tiles = {}
                oh_tiles = {}
                for b in range(B):
                    st = {}
                    for cc in range(2):
                        t = stile([128, 512], "st", bufs=4)
                        dma(t, supp_t[b, cc, :, ic8 * 512:(ic8 + 1) * 512])
                        st[cc] = t
                    ksT = {}
                    for mo in range(2):
                        ps = wps()
                        for cc in range(2):
                            mm(ps, wks_sb[:, cc * 256 + mo * 128:
                                          cc * 256 + mo * 128 + 128],
                               st[cc], start=(cc == 0), stop=(cc == 1))
                        kt = stile([128, 512], "ksT", bufs=4)
                        nc.vector.tensor_scalar(kt, ps, scalar1=bks_c[:, mo:mo + 1],
                                                scalar2=None, op0=OP.add)
                        ksT[mo] = kt
                    # AS chunk [a, 512] (scaled via qaT)
                    psa = wps()
                    for mo in range(2):
                        mm(psa, qaT[:, (b * 2 + mo) * NA:(b * 2 + mo + 1) * NA],
                           ksT[mo], start=(mo == 0), stop=(mo == 1))
                    asb = stile([128, 512], "AS", bufs=3)
                    nc.vector.tensor_copy(asb, psa)
                    as_tiles[b] = asb
                    for sub in range(4):
                        # vs tile [i=128, c=256]
                        psv = wps((128, C))
                        for cc in range(2):
                            mm(psv, st[cc][:, sub * 128:(sub + 1) * 128],
                               wvs_sb[:, cc * 256:(cc + 1) * 256],
                               start=(cc == 0), stop=(cc == 1))
                        vt = stile([128, C], "vs", bufs=10)
                        nc.vector.tensor_add(vt, psv, bvsb)
                        vs_tiles[(b, sub)] = vt
                        # AS^T tile for argmax
                        psat = wps((128, 128))
                        for mo in range(2):
                            mm(psat, ksT[mo][:, sub * 128:(sub + 1) * 128],
                               qaT[:, (b * 2 + mo) * NA:(b * 2 + mo + 1) * NA],
                               start=(mo == 0), stop=(mo == 1))
                        ast = stile([128, 128], "ast")
                        nc.vector.tensor_copy(ast, psat)
                        mx = stile([128, 8], "mxa")
                        nc.vector.max(mx, ast)
                        idx = stile([128, 8], "idxa", dtype=u32)
                        nc.vector.max_index(idx, mx, ast)
                        acol = stile([128, 1], "acol")
                        nc.vector.tensor_copy(acol, idx[:, 0:1])
                        oht = stile([128, 128], "oht")
                        nc.vector.tensor_scalar(oht, iota_f, scalar1=acol,
                                                scalar2=None, op0=OP.is_equal)
                        psoh = wps((128, 128))
                        nc.tensor.transpose(psoh, oht, ident)
                        ohsb = stile([128, 128], "ohA", bufs=10)
                        nc.vector.tensor_copy(ohsb, psoh)
                        oh_tiles[(b, sub)] = ohsb
                for sub in range(4):
                    it = ic8 * 4 + sub
                    psd = wps()
                    mm(psd, as_tiles[0][:, sub * 128:(sub + 1) * 128],
                       QAT[:, :JS], start=True, stop=False)
                    mm(psd, as_tiles[1][:, sub * 128:(sub + 1) * 128],
                       QAT[:, JS:], start=False, stop=False)
                    mm(psd, oh_tiles[(0, sub)], Vq[:, :JS], start=False, stop=False)
                    mm(psd, oh_tiles[(1, sub)], Vq[:, JS:], start=False, stop=True)
                    p0 = stile([128, JS], "p0")
                    nc.scalar.activation(p0, psd, AF.Sigmoid)
                    for b in range(B):
                        for mo in range(2):
                            mm(dec_ps[b * 2 + mo],
                               vs_tiles[(b, sub)][:, mo * 128:(mo + 1) * 128],
                               p0, start=(it == 0), stop=(it == 31),
                               skip_group_check=True)

            # ---------------- FFN ----------------
            decT = ptile([128, 4 * JS], "decT")    # (b, mo) chunks [c, j]
            for mo in range(2):
                nc.vector.tensor_copy(decT[:, mo * JS:(mo + 1) * JS], dec_ps[mo])
                nc.vector.tensor_scalar(
                    decT[:, (2 + mo) * JS:(3 + mo) * JS], dec_ps[2 + mo],
                    scalar1=csv1[:, mo:mo + 1], scalar2=-1.0,
                    op0=OP.subtract, op1=OP.mult)
            dec2_ps = [ppd.tile([128, JS], f32, name=f"dec2{q}", tag="dec", bufs=4)
                       for q in range(4)]
            for fo in range(16):
                w1t = stile([128, 256], "w1t", bufs=4)
                for cc in range(2):
                    dma(w1t[:, cc * 128:(cc + 1) * 128],
                        w1_d[cc, :, fo * 128:(fo + 1) * 128])
                w2t = stile([128, 256], "w2t", bufs=4)
                dma(w2t, w2_d[fo])
                for b in range(B):
                    psh = wps()
                    for cc in range(2):
                        mm(psh, w1t[:, cc * 128:(cc + 1) * 128],
                           decT[:, (b * 2 + cc) * JS:(b * 2 + cc + 1) * JS],
                           start=(cc == 0), stop=(cc == 1))
                    hsb = stile([128, JS], "hfo")
                    nc.scalar.activation(hsb, psh, AF.Relu, bias=b1_sb[:, fo:fo + 1])
                    for mo in range(2):
                        mm(dec2_ps[b * 2 + mo], w2t[:, mo * 128:(mo + 1) * 128],
                           hsb, start=(fo == 0), stop=(fo == 15),
                           skip_group_check=True)
            dec2T = ptile([128, 4 * JS], "dec2T")  # (b, mo) chunks [c, j]
            for b in range(B):
                for mo in range(2):
                    nc.vector.tensor_scalar(
                        dec2T[:, (b * 2 + mo) * JS:(b * 2 + mo + 1) * JS],
                        dec2_ps[b * 2 + mo], scalar1=b2_sb[:, mo:mo + 1],
                        scalar2=None, op0=OP.add)

            # ---------------- transpose to [j, c] and write x ----------------
            for b in range(B):
                for jt in range(4):
                    d2j = stile([128, C], "d2j")
                    for mo in range(2):
                        pst = wps((128, 128))
                        nc.tensor.transpose(
                            pst,
                            dec2T[:, (b * 2 + mo) * JS + jt * 128:
                                  (b * 2 + mo) * JS + jt * 128 + 128], ident)
                        nc.vector.tensor_copy(d2j[:, mo * 128:(mo + 1) * 128], pst)
                    dst = x_dram[b, jt * 128 * C:(jt + 1) * 128 * C]
                    dst = dst.rearrange("(p c) -> p c", c=C)
                    dma(dst, d2j)

            # ---------------- A2A exchange ----------------
            for m in range(NCORES):
                lo = max(0, 8 * m - 2)
                hi = min(H, 8 * m + 10)
                r0 = lo - (8 * m - 2)
                nr = hi - lo
                for b in range(B):
                    xv = x_dram[b].rearrange("(cv s) -> cv s", s=H * W)
                    dma(a2a_in[m, b, :, r0:r0 + nr, :],
                        xv[:, lo * W:(lo + nr) * W])
                    if m == 0:
                        dma(a2a_in[m, b, :, 0:2, :], zeros_sb)
                    if m == NCORES - 1:
                        dma(a2a_in[m, b, :, 10:12, :], zeros_sb)
            nc.gpsimd.collective_compute(
                "AllToAll", OP.bypass,
                replica_groups=[list(range(NCORES))],
                ins=[a2a_in], outs=[a2a_out])

            # ---------------- conv ----------------
            for b in range(B):
                xpad = {}
                for cc in range(2):
                    xp = stile([128, 12 * 66], "xpad", bufs=4)
                    nc.gpsimd.memset(xp, 0.0)
                    xpv = xp.rearrange("p (r c) -> p r c", c=66)
                    for rr in range(4):
                        r = cc * 4 + rr
                        dma(xpv[rr * 32:(rr + 1) * 32, :, 1:65], a2a_out[r, b])
                    xpad[cc] = xp
                c3p = stile([32, 10 * 66], "c3p", bufs=2)
                nc.gpsimd.memset(c3p, 0.0)
                c3pv = c3p.rearrange("p (r c) -> p r c", c=66)
                for half in range(2):
                    ps3 = wps((32, 320))
                    first = True
                    for kh in range(3):
                        for kw in range(3):
                            for cc in range(2):
                                xpv = xpad[cc].rearrange("p (r c) -> p r c", c=66)
                                rhs = xpv[:, half * 5 + kh: half * 5 + kh + 5,
                                          kw:kw + 64]
                                mm(ps3, w3_sb[:, ((kh * 3 + kw) * 2 + cc) * 32:
                                              ((kh * 3 + kw) * 2 + cc + 1) * 32],
                                   rhs, start=first,
                                   stop=(kh == 2 and kw == 2 and cc == 1))
                                first = False
                    dstv = c3pv[:, half * 5:half * 5 + 5, 1:65]
                    nc.scalar.activation(dstv, ps3, AF.Relu)
                    nc.vector.tensor_mul(dstv, dstv,
                                         rowmask[:, half * 320:(half + 1) * 320])
                ps1 = wps((3, 512))
                first = True
                for kh in range(3):
                    for kw in range(3):
                        rhs = c3pv[:, kh:kh + 8, kw:kw + 64]
                        mm(ps1, w1c_sb[:, (kh * 3 + kw) * 3:(kh * 3 + kw + 1) * 3],
                           rhs, start=first, stop=(kh == 2 and kw == 2))
                        first = False
                osb = stile([3, 512], "osb")
                nc.vector.tensor_copy(osb, ps1)
                dma(out_ext[b], osb.rearrange("p (r c) -> p r c", c=64))

    nc.compile()
    return nc


def _get_nc():
    if "nc" not in _CACHE:
        _CACHE["nc"] = _build_nc()
    return _CACHE["nc"]


def _chunk2(v):
    """[256] bias -> [128, 2] column-per-chunk."""
    return np.stack([v[:128], v[128:]], axis=1).astype(np.float32)


def _prep_inputs(tok_agent, enc_feat_supp, enc_feat_query,
                 Wqa, bqa, Wks, bks, Wka, bka, Wvs, bvs,
                 W1, b1, W2, b2, conv3_w, conv1_w):
    tok, supp, query = tok_agent, enc_feat_supp, enc_feat_query
    base = {}
    base["supp_t"] = np.ascontiguousarray(
        supp.transpose(0, 2, 1).reshape(B, 2, 128, HW)).astype(np.float32)
    base["tok_t"] = np.ascontiguousarray(
        tok.transpose(0, 2, 1).reshape(B, 2, 128, NA)).astype(np.float32)
    base["wq"] = np.ascontiguousarray(
        Wqa.reshape(2, 128, 256).transpose(1, 0, 2).reshape(128, 512))
    base["wk"] = np.ascontiguousarray(
        Wka.reshape(2, 128, 256).transpose(1, 0, 2).reshape(128, 512))
    base["wks"] = np.ascontiguousarray(
        Wks.reshape(2, 128, 256).transpose(1, 0, 2).reshape(128, 512))
    base["wvs"] = np.ascontiguousarray(
        Wvs.reshape(2, 128, 256).transpose(1, 0, 2).reshape(128, 512))
    base["w1"] = np.ascontiguousarray(W1.reshape(2, 128, DFF))
    base["w2"] = np.ascontiguousarray(W2.reshape(16, 128, C))
    base["w3"] = np.ascontiguousarray(
        conv3_w.transpose(2, 3, 1, 0).reshape(3, 3, 2, 128, 32)
        .transpose(3, 0, 1, 2, 4).reshape(128, 576))
    base["w1c"] = np.ascontiguousarray(
        conv1_w.transpose(2, 3, 1, 0).transpose(2, 0, 1, 3).reshape(32, 27))
    base["bqa_s"] = _chunk2(bqa * SCALE)
    base["bka_s"] = _chunk2(bka * SCALE)
    base["bqa_c"] = _chunk2(bqa)
    base["bks_c"] = _chunk2(bks)
    base["bvs_bcast"] = np.tile(bvs[None, :], (128, 1)).astype(np.float32)
    base["b1c"] = np.ascontiguousarray(b1.reshape(16, 128).T).astype(np.float32)
    base["b2c"] = _chunk2(b2)
    cs_vs1 = supp[1].sum(axis=0, dtype=np.float64) @ Wvs.astype(np.float64) \
        + HW * bvs.astype(np.float64)
    base["cs_vs1"] = _chunk2(cs_vs1.astype(np.float32))

    in_maps = []
    for k in range(NCORES):
        m = dict(base)
        m["query_t"] = np.ascontiguousarray(
            query[:, k * JS:(k + 1) * JS].transpose(0, 2, 1)
            .reshape(B, 2, 128, JS)).astype(np.float32)
        rmask = np.array([1.0 if 0 <= 8 * k - 1 + lr < H else 0.0
                          for lr in range(10)], np.float32)
        m["rowmask"] = np.ascontiguousarray(
            np.tile(np.repeat(rmask, W)[None, :], (32, 1))).astype(np.float32)
        in_maps.append(m)
    return in_maps


def run(in_maps, trace=False):
    from concourse.bass_utils import run_bass_kernel_spmd
    nc = _get_nc()
    return run_bass_kernel_spmd(nc, in_maps, core_ids=list(range(NCORES)),
                                trace=trace)


def kernel(**inputs):
    inputs = {k: np.asarray(v, dtype=np.float32) for k, v in inputs.items()}
    in_maps = _prep_inputs(**inputs)
    res = run(in_maps, trace=False)
    return np.concatenate([res.results[k]["out"] for k in range(NCORES)], axis=2)


if __name__ == "__main__":
    print("building...")
    _get_nc()
    print("built ok")


# revision 12
# speedup vs baseline: 1.4300x; 1.4300x over previous
"""AgentMatchingDecoder Trainium2 kernel — 8-core SPMD, query(j)-sharded.

Math: with B=2, softmax over the batch axis collapses to a sigmoid:
  p0 = sigmoid(d), p1 = 1-p0,  d[i,j] = l0[i,j]-l1[i,j]
  d = sum_a' U[a',i] V[a',j]   (K=512 extended contraction)
  U = [AS0; AS1; onehotA0; onehotA1]      ([a,i] layouts)
  V = [QA0^T; -QA1^T; 1e6*ohQ0^T; -1e6*ohQ1^T]
dec1 uses the identity sum_i (1-p0) vs1 = colsum(vs1) - sum_i p0 vs1,
with colsum(vs1) computed on the host from the raw inputs.
Each core owns j in [512k, 512k+512) -> conv-input channels [32k,32k+32);
an AllToAll exchanges channel blocks for spatial halo rows so core k
computes output rows [8k, 8k+8).
"""
import os
import sys

import numpy as np

sys.path.insert(0, "/opt/trn_rl_repo")

B, NA, HW, C = 2, 128, 4096, 256
DFF, H, W = 2048, 64, 64
NCORES = 8
JS = HW // NCORES           # 512
RS = H // NCORES            # 8
SCALE = 1.0 / np.sqrt(C // 8)
BIG = 1.0e6

_CACHE = {}


def _build_nc():
    import concourse.bass as bass
    import concourse.mybir as mybir
    from concourse import bacc, tile
    from concourse.masks import make_identity

    f32 = mybir.dt.float32
    bf16 = mybir.dt.bfloat16
    u32 = mybir.dt.uint32
    i32 = mybir.dt.int32
    AF = mybir.ActivationFunctionType
    OP = mybir.AluOpType

    nc = bacc.Bacc("TRN2", target_bir_lowering=False, debug=False,
                   num_devices=NCORES)

    def inp(name, shape, dtype=f32):
        return nc.dram_tensor(name, list(shape), dtype, kind="ExternalInput").ap()

    supp_t = inp("supp_t", [B, 2, 128, HW])
    query_t = inp("query_t", [B, 2, 128, JS])
    tok_t = inp("tok_t", [B, 2, 128, NA])
    wq_d = inp("wq", [128, 512])
    wk_d = inp("wk", [128, 512])
    wks_d = inp("wks", [128, 512])
    wvs_d = inp("wvs", [128, 512], bf16)
    w1_d = inp("w1", [2, 128, DFF], bf16)
    w2_d = inp("w2", [16, 128, C], bf16)
    w3_d = inp("w3", [128, 576], bf16)
    w1c_d = inp("w1c", [32, 27], bf16)
    bqa_s_d = inp("bqa_s", [128, 2])    # scaled bqa (for qa^T)
    bka_s_d = inp("bka_s", [128, 2])    # scaled bka
    bqa_c_d = inp("bqa_c", [128, 2])    # unscaled bqa (for qq^T)
    bks_c_d = inp("bks_c", [128, 2])
    bvsb_d = inp("bvs_bcast", [128, C])
    b1_d = inp("b1c", [128, 16])
    b2_d = inp("b2c", [128, 2])
    csv1_d = inp("cs_vs1", [128, 2])
    rowmask_d = inp("rowmask", [32, 640], bf16)

    out_ext = nc.dram_tensor("out", [B, 3, RS, W], f32, kind="ExternalOutput").ap()
    dbg_p0 = nc.dram_tensor("dbg_p0", [128, JS], bf16, kind="ExternalOutput").ap()
    dbg_vs = nc.dram_tensor("dbg_vs", [128, C], bf16, kind="ExternalOutput").ap()
    dbg_decT = nc.dram_tensor("dbg_decT", [128, 4 * JS], bf16, kind="ExternalOutput").ap()
    dbg_dec2T = nc.dram_tensor("dbg_dec2T", [128, 4 * JS], f32, kind="ExternalOutput").ap()
    dbg_h = nc.dram_tensor("dbg_h", [128, JS], bf16, kind="ExternalOutput").ap()
    dbg_d = nc.dram_tensor("dbg_d", [128, JS], f32, kind="ExternalOutput").ap()

    # internal DRAM
    x_dram = nc.dram_tensor("x_scratch", [B, 32 * H * W], bf16).ap()
    a2a_in = nc.dram_tensor("a2a_in", [NCORES, B, 32, 12, W], bf16).ap()
    a2a_out = nc.dram_tensor("a2a_out", [NCORES, B, 32, 12, W], bf16).ap()

    with tile.TileContext(nc) as tc:
        with (
            tc.tile_pool(name="const", bufs=1) as cpool,
            tc.tile_pool(name="pers", bufs=1) as pers,
            tc.tile_pool(name="stream", bufs=3) as spool,
            tc.tile_pool(name="psw", bufs=4, space="PSUM") as ppw,
            tc.tile_pool(name="psdec", bufs=4, space="PSUM") as ppd,
        ):
            def ctile(shape, name, dtype=f32):
                return cpool.tile(shape, dtype, name=name, tag=name)

            def ptile(shape, name, dtype=f32):
                return pers.tile(shape, dtype, name=name, tag=name)

            def stile(shape, tag, dtype=f32, bufs=3):
                return spool.tile(shape, dtype, name=tag, tag=tag, bufs=bufs)

            def wps(shape=(128, 512)):
                return ppw.tile(list(shape), f32, name="w", tag="w", bufs=4)

            dma = nc.sync.dma_start
            mm = nc.tensor.matmul

            # ---------------- constants ----------------
            ident = ctile([128, 128], "ident")
            make_identity(nc, ident)
            iota_i = ctile([128, 128], "iota_i", i32)
            nc.gpsimd.iota(iota_i, pattern=[[1, 128]], base=0, channel_multiplier=0)
            iota_f = ctile([128, 128], "iota_f")
            nc.vector.tensor_copy(iota_f, iota_i)

            wq_sb = ctile([128, 512], "wq_sb"); dma(wq_sb, wq_d)
            wk_sb = ctile([128, 512], "wk_sb"); dma(wk_sb, wk_d)
            wks_sb = ctile([128, 512], "wks_sb"); dma(wks_sb, wks_d)
            wvs_sb = ctile([128, 512], "wvs_sb", bf16); dma(wvs_sb, wvs_d)
            w3_sb = ctile([128, 576], "w3_sb", bf16); dma(w3_sb, w3_d)
            w1c_sb = ctile([32, 27], "w1c_sb", bf16); dma(w1c_sb, w1c_d)
            bqa_s = ctile([128, 2], "bqa_s_sb"); dma(bqa_s, bqa_s_d)
            bka_s = ctile([128, 2], "bka_s_sb"); dma(bka_s, bka_s_d)
            bqa_c = ctile([128, 2], "bqa_c_sb"); dma(bqa_c, bqa_c_d)
            bks_c = ctile([128, 2], "bks_c_sb"); dma(bks_c, bks_c_d)
            bvsb = ctile([128, C], "bvsb_sb"); dma(bvsb, bvsb_d)
            b1_sb = ctile([128, 16], "b1_sb"); dma(b1_sb, b1_d)
            b2_sb = ctile([128, 2], "b2_sb"); dma(b2_sb, b2_d)
            csv1 = ctile([128, 2], "csv1_sb"); dma(csv1, csv1_d)
            rowmask = ctile([32, 640], "rowmask_sb", bf16); dma(rowmask, rowmask_d)
            zeros_sb = ctile([32, 128], "zeros_sb", bf16)
            nc.gpsimd.memset(zeros_sb, 0.0)

            tok_sb = ctile([128, 4 * NA], "tok_sb")
            for b in range(B):
                for cc in range(2):
                    dma(tok_sb[:, (b * 2 + cc) * NA:(b * 2 + cc + 1) * NA],
                        tok_t[b, cc])
            qt_sb = ctile([128, 4 * JS], "qt_sb")
            for b in range(B):
                for cc in range(2):
                    dma(qt_sb[:, (b * 2 + cc) * JS:(b * 2 + cc + 1) * JS],
                        query_t[b, cc])

            # ---------------- phase 1: qa^T, ka^T, qq^T ----------------
            qaT = ptile([128, 4 * NA], "qaT")      # (b, mo) chunks, scaled
            kaT = ptile([128, 4 * NA], "kaT")
            qqT = ptile([128, 4 * JS], "qqT")      # (b, cc') chunks
            for b in range(B):
                for mo in range(2):
                    ps = wps((128, NA))
                    for cc in range(2):
                        mm(ps, wq_sb[:, cc * 256 + mo * 128: cc * 256 + mo * 128 + 128],
                           tok_sb[:, (b * 2 + cc) * NA:(b * 2 + cc + 1) * NA],
                           start=(cc == 0), stop=(cc == 1))
                    nc.vector.tensor_scalar(
                        qaT[:, (b * 2 + mo) * NA:(b * 2 + mo + 1) * NA], ps,
                        scalar1=SCALE, scalar2=bqa_s[:, mo:mo + 1],
                        op0=OP.mult, op1=OP.add)
                    ps2 = wps((128, NA))
                    for cc in range(2):
                        mm(ps2, wk_sb[:, cc * 256 + mo * 128: cc * 256 + mo * 128 + 128],
                           tok_sb[:, (b * 2 + cc) * NA:(b * 2 + cc + 1) * NA],
                           start=(cc == 0), stop=(cc == 1))
                    nc.vector.tensor_scalar(
                        kaT[:, (b * 2 + mo) * NA:(b * 2 + mo + 1) * NA], ps2,
                        scalar1=SCALE, scalar2=bka_s[:, mo:mo + 1],
                        op0=OP.mult, op1=OP.add)
                    ps3 = wps()
                    for cc in range(2):
                        mm(ps3, wq_sb[:, cc * 256 + mo * 128: cc * 256 + mo * 128 + 128],
                           qt_sb[:, (b * 2 + cc) * JS:(b * 2 + cc + 1) * JS],
                           start=(cc == 0), stop=(cc == 1))
                    nc.vector.tensor_scalar(
                        qqT[:, (b * 2 + mo) * JS:(b * 2 + mo + 1) * JS], ps3,
                        scalar1=bqa_c[:, mo:mo + 1], scalar2=None, op0=OP.add)

            # ---------------- phase 2: QA^T (V rows), Q argmax, ohQ ----------------
            QAT = ptile([128, 2 * JS], "QAT")      # [a, j] per b (b1 negated)
            Vq = ptile([128, 2 * JS], "Vq")        # [a, j] per b: +-1e6 one-hot(Q)
            for b in range(B):
                ps = wps()
                for cc in range(2):
                    mm(ps, kaT[:, (b * 2 + cc) * NA:(b * 2 + cc + 1) * NA],
                       qqT[:, (b * 2 + cc) * JS:(b * 2 + cc + 1) * JS],
                       start=(cc == 0), stop=(cc == 1))
                if b == 0:
                    nc.vector.tensor_copy(QAT[:, :JS], ps)
                else:
                    nc.vector.tensor_scalar(QAT[:, JS:], ps, scalar1=-1.0,
                                            scalar2=None, op0=OP.mult)
                for jt in range(4):
                    psq = wps((128, NA))
                    for cc in range(2):
                        mm(psq, qqT[:, (b * 2 + cc) * JS + jt * 128:
                                    (b * 2 + cc) * JS + jt * 128 + 128],
                           kaT[:, (b * 2 + cc) * NA:(b * 2 + cc + 1) * NA],
                           start=(cc == 0), stop=(cc == 1))
                    qat_sb = stile([128, NA], "qat_sb")
                    nc.vector.tensor_copy(qat_sb, psq)
                    mx = stile([128, 8], "mxq")
                    nc.vector.max(mx, qat_sb)
                    idx = stile([128, 8], "idxq", dtype=u32)
                    nc.vector.max_index(idx, mx, qat_sb)
                    qcol = stile([128, 1], "qcol")
                    nc.vector.tensor_copy(qcol, idx[:, 0:1])
                    ohqt = stile([128, 128], "ohqt")
                    nc.vector.tensor_scalar(ohqt, iota_f, scalar1=qcol,
                                            scalar2=(BIG if b == 0 else -BIG),
                                            op0=OP.is_equal, op1=OP.mult)
                    pst = wps((128, 128))
                    nc.tensor.transpose(pst, ohqt, ident)
                    nc.vector.tensor_copy(
                        Vq[:, b * JS + jt * 128: b * JS + jt * 128 + 128], pst)

            # ---------------- main fused loop over support tiles ----------------
            dec_ps = [ppd.tile([128, JS], f32, name=f"dec{q}", tag="dec", bufs=4)
                      for q in range(4)]           # (b, mo)
            for ic8 in range(8):
                as_tiles = {}
                vs_tiles = {}
                oh_tiles = {}
                for b in range(B):
                    st = {}
                    for cc in range(2):
                        t = stile([128, 512], "st", bufs=4)
                        dma(t, supp_t[b, cc, :, ic8 * 512:(ic8 + 1) * 512])
                        st[cc] = t
                    stb = {}
                    for cc in range(2):
                        tb = stile([128, 512], "stb", dtype=bf16, bufs=4)
                        nc.vector.tensor_copy(tb, st[cc])
                        stb[cc] = tb
                    ksT = {}
                    for mo in range(2):
                        ps = wps()
                        for cc in range(2):
                            mm(ps, wks_sb[:, cc * 256 + mo * 128:
                                          cc * 256 + mo * 128 + 128],
                               st[cc], start=(cc == 0), stop=(cc == 1))
                        kt = stile([128, 512], "ksT", bufs=4)
                        nc.vector.tensor_scalar(kt, ps, scalar1=bks_c[:, mo:mo + 1],
                                                scalar2=None, op0=OP.add)
                        ksT[mo] = kt
                    # AS chunk [a, 512] (scaled via qaT)
                    psa = wps()
                    for mo in range(2):
                        mm(psa, qaT[:, (b * 2 + mo) * NA:(b * 2 + mo + 1) * NA],
                           ksT[mo], start=(mo == 0), stop=(mo == 1))
                    asb = stile([128, 512], "AS", bufs=3)
                    nc.vector.tensor_copy(asb, psa)
                    as_tiles[b] = asb
                    for sub in range(4):
                        # vs tile [i=128, c=256]
                        psv = wps((128, C))
                        for cc in range(2):
                            mm(psv, stb[cc][:, sub * 128:(sub + 1) * 128],
                               wvs_sb[:, cc * 256:(cc + 1) * 256],
                               start=(cc == 0), stop=(cc == 1))
                        vt = stile([128, C], "vs", dtype=bf16, bufs=10)
                        nc.vector.tensor_add(vt, psv, bvsb)
                        if ic8 == 7 and b == 0 and sub == 3:
                            dma(dbg_vs, vt)
                        vs_tiles[(b, sub)] = vt
                        # AS^T tile for argmax
                        psat = wps((128, 128))
                        for mo in range(2):
                            mm(psat, ksT[mo][:, sub * 128:(sub + 1) * 128],
                               qaT[:, (b * 2 + mo) * NA:(b * 2 + mo + 1) * NA],
                               start=(mo == 0), stop=(mo == 1))
                        ast = stile([128, 128], "ast")
                        nc.vector.tensor_copy(ast, psat)
                        mx = stile([128, 8], "mxa")
                        nc.vector.max(mx, ast)
                        idx = stile([128, 8], "idxa", dtype=u32)
                        nc.vector.max_index(idx, mx, ast)
                        acol = stile([128, 1], "acol")
                        nc.vector.tensor_copy(acol, idx[:, 0:1])
                        oht = stile([128, 128], "oht")
                        nc.vector.tensor_scalar(oht, iota_f, scalar1=acol,
                                                scalar2=None, op0=OP.is_equal)
                        psoh = wps((128, 128))
                        nc.tensor.transpose(psoh, oht, ident)
                        ohsb = stile([128, 128], "ohA", bufs=10)
                        nc.vector.tensor_copy(ohsb, psoh)
                        oh_tiles[(b, sub)] = ohsb
                for sub in range(4):
                    it = ic8 * 4 + sub
                    psd = wps()
                    mm(psd, as_tiles[0][:, sub * 128:(sub + 1) * 128],
                       QAT[:, :JS], start=True, stop=False)
                    mm(psd, as_tiles[1][:, sub * 128:(sub + 1) * 128],
                       QAT[:, JS:], start=False, stop=False)
                    mm(psd, oh_tiles[(0, sub)], Vq[:, :JS], start=False, stop=False)
                    mm(psd, oh_tiles[(1, sub)], Vq[:, JS:], start=False, stop=True)
                    p0 = stile([128, JS], "p0", dtype=bf16)
                    if it == 31:
                        dtmp = stile([128, JS], "dtmp")
                        nc.vector.tensor_copy(dtmp, psd)
                        dma(dbg_d, dtmp)
                    nc.scalar.activation(p0, psd, AF.Sigmoid)
                    if it == 31:
                        dma(dbg_p0, p0)
                    for b in range(B):
                        for mo in range(2):
                            mm(dec_ps[b * 2 + mo],
                               vs_tiles[(b, sub)][:, mo * 128:(mo + 1) * 128],
                               p0, start=(it == 0), stop=(it == 31),
                               skip_group_check=True)

            # ---------------- FFN ----------------
            decT = ptile([128, 4 * JS], "decT", bf16)    # (b, mo) chunks [c, j]
            for mo in range(2):
                nc.vector.tensor_copy(decT[:, mo * JS:(mo + 1) * JS], dec_ps[mo])
                nc.vector.tensor_scalar(
                    decT[:, (2 + mo) * JS:(3 + mo) * JS], dec_ps[2 + mo],
                    scalar1=csv1[:, mo:mo + 1], scalar2=-1.0,
                    op0=OP.subtract, op1=OP.mult)
            dma(dbg_decT, decT)
            dec2_ps = [ppd.tile([128, JS], f32, name=f"dec2{q}", tag="dec", bufs=4)
                       for q in range(4)]
            for fo in range(16):
                w1t = stile([128, 256], "w1t", dtype=bf16, bufs=4)
                for cc in range(2):
                    dma(w1t[:, cc * 128:(cc + 1) * 128],
                        w1_d[cc, :, fo * 128:(fo + 1) * 128])
                w2t = stile([128, 256], "w2t", dtype=bf16, bufs=4)
                dma(w2t, w2_d[fo])
                for b in range(B):
                    psh = wps()
                    for cc in range(2):
                        mm(psh, w1t[:, cc * 128:(cc + 1) * 128],
                           decT[:, (b * 2 + cc) * JS:(b * 2 + cc + 1) * JS],
                           start=(cc == 0), stop=(cc == 1))
                    hsb = stile([128, JS], "hfo", dtype=bf16)
                    nc.scalar.activation(hsb, psh, AF.Relu, bias=b1_sb[:, fo:fo + 1])
                    if fo == 15 and b == 0:
                        dma(dbg_h, hsb)
                    for mo in range(2):
                        mm(dec2_ps[b * 2 + mo], w2t[:, mo * 128:(mo + 1) * 128],
                           hsb, start=(fo == 0), stop=(fo == 15),
                           skip_group_check=True)
            dec2T = ptile([128, 4 * JS], "dec2T")  # (b, mo) chunks [c, j]
            for b in range(B):
                for mo in range(2):
                    nc.vector.tensor_scalar(
                        dec2T[:, (b * 2 + mo) * JS:(b * 2 + mo + 1) * JS],
                        dec2_ps[b * 2 + mo], scalar1=b2_sb[:, mo:mo + 1],
                        scalar2=None, op0=OP.add)

            dma(dbg_dec2T, dec2T)
            # ---------------- transpose to [j, c] and write x ----------------
            for b in range(B):
                for jt in range(4):
                    d2j = stile([128, C], "d2j", dtype=bf16)
                    for mo in range(2):
                        pst = wps((128, 128))
                        nc.tensor.transpose(
                            pst,
                            dec2T[:, (b * 2 + mo) * JS + jt * 128:
                                  (b * 2 + mo) * JS + jt * 128 + 128], ident)
                        nc.vector.tensor_copy(d2j[:, mo * 128:(mo + 1) * 128], pst)
                    dst = x_dram[b, jt * 128 * C:(jt + 1) * 128 * C]
                    dst = dst.rearrange("(p c) -> p c", c=C)
                    dma(dst, d2j)

            # ---------------- A2A exchange ----------------
            for m in range(NCORES):
                lo = max(0, 8 * m - 2)
                hi = min(H, 8 * m + 10)
                r0 = lo - (8 * m - 2)
                nr = hi - lo
                for b in range(B):
                    xv = x_dram[b].rearrange("(cv s) -> cv s", s=H * W)
                    dma(a2a_in[m, b, :, r0:r0 + nr, :],
                        xv[:, lo * W:(lo + nr) * W])
                    if m == 0:
                        dma(a2a_in[m, b, :, 0:2, :], zeros_sb)
                    if m == NCORES - 1:
                        dma(a2a_in[m, b, :, 10:12, :], zeros_sb)
            nc.gpsimd.collective_compute(
                "AllToAll", OP.bypass,
                replica_groups=[list(range(NCORES))],
                ins=[a2a_in], outs=[a2a_out])

            # ---------------- conv ----------------
            for b in range(B):
                xpad = {}
                for cc in range(2):
                    xp = stile([128, 12 * 66], "xpad", dtype=bf16, bufs=4)
                    nc.gpsimd.memset(xp, 0.0)
                    xpv = xp.rearrange("p (r c) -> p r c", c=66)
                    for rr in range(4):
                        r = cc * 4 + rr
                        dma(xpv[rr * 32:(rr + 1) * 32, :, 1:65], a2a_out[r, b])
                    xpad[cc] = xp
                c3p = stile([32, 10 * 66], "c3p", dtype=bf16, bufs=2)
                nc.gpsimd.memset(c3p, 0.0)
                c3pv = c3p.rearrange("p (r c) -> p r c", c=66)
                for half in range(2):
                    ps3 = wps((32, 320))
                    first = True
                    for kh in range(3):
                        for kw in range(3):
                            for cc in range(2):
                                xpv = xpad[cc].rearrange("p (r c) -> p r c", c=66)
                                rhs = xpv[:, half * 5 + kh: half * 5 + kh + 5,
                                          kw:kw + 64]
                                mm(ps3, w3_sb[:, ((kh * 3 + kw) * 2 + cc) * 32:
                                              ((kh * 3 + kw) * 2 + cc + 1) * 32],
                                   rhs, start=first,
                                   stop=(kh == 2 and kw == 2 and cc == 1))
                                first = False
                    dstv = c3pv[:, half * 5:half * 5 + 5, 1:65]
                    nc.scalar.activation(dstv, ps3, AF.Relu)
                    nc.vector.tensor_mul(dstv, dstv,
                                         rowmask[:, half * 320:(half + 1) * 320])
                ps1 = wps((3, 512))
                first = True
                for kh in range(3):
                    for kw in range(3):
                        rhs = c3pv[:, kh:kh + 8, kw:kw + 64]
                        mm(ps1, w1c_sb[:, (kh * 3 + kw) * 3:(kh * 3 + kw + 1) * 3],
                           rhs, start=first, stop=(kh == 2 and kw == 2))
                        first = False
                osb = stile([3, 512], "osb")
                nc.vector.tensor_copy(osb, ps1)
                dma(out_ext[b], osb.rearrange("p (r c) -> p r c", c=64))

    nc.compile()
    return nc


def _get_nc():
    if "nc" not in _CACHE:
        _CACHE["nc"] = _build_nc()
    return _CACHE["nc"]


def _chunk2(v):
    """[256] bias -> [128, 2] column-per-chunk."""
    return np.stack([v[:128], v[128:]], axis=1).astype(np.float32)


def _prep_inputs(tok_agent, enc_feat_supp, enc_feat_query,
                 Wqa, bqa, Wks, bks, Wka, bka, Wvs, bvs,
                 W1, b1, W2, b2, conv3_w, conv1_w):
    tok, supp, query = tok_agent, enc_feat_supp, enc_feat_query
    base = {}
    base["supp_t"] = np.ascontiguousarray(
        supp.transpose(0, 2, 1).reshape(B, 2, 128, HW)).astype(np.float32)
    base["tok_t"] = np.ascontiguousarray(
        tok.transpose(0, 2, 1).reshape(B, 2, 128, NA)).astype(np.float32)
    base["wq"] = np.ascontiguousarray(
        Wqa.reshape(2, 128, 256).transpose(1, 0, 2).reshape(128, 512))
    base["wk"] = np.ascontiguousarray(
        Wka.reshape(2, 128, 256).transpose(1, 0, 2).reshape(128, 512))
    base["wks"] = np.ascontiguousarray(
        Wks.reshape(2, 128, 256).transpose(1, 0, 2).reshape(128, 512))
    base["wvs"] = np.ascontiguousarray(
        Wvs.reshape(2, 128, 256).transpose(1, 0, 2).reshape(128, 512))
    base["w1"] = np.ascontiguousarray(W1.reshape(2, 128, DFF))
    base["w2"] = np.ascontiguousarray(W2.reshape(16, 128, C))
    base["w3"] = np.ascontiguousarray(
        conv3_w.transpose(2, 3, 1, 0).reshape(3, 3, 2, 128, 32)
        .transpose(3, 0, 1, 2, 4).reshape(128, 576))
    base["w1c"] = np.ascontiguousarray(
        conv1_w.transpose(2, 3, 1, 0).transpose(2, 0, 1, 3).reshape(32, 27))
    base["bqa_s"] = _chunk2(bqa * SCALE)
    base["bka_s"] = _chunk2(bka * SCALE)
    base["bqa_c"] = _chunk2(bqa)
    base["bks_c"] = _chunk2(bks)
    base["bvs_bcast"] = np.tile(bvs[None, :], (128, 1)).astype(np.float32)
    base["b1c"] = np.ascontiguousarray(b1.reshape(16, 128).T).astype(np.float32)
    base["b2c"] = _chunk2(b2)
    cs_vs1 = supp[1].sum(axis=0, dtype=np.float64) @ Wvs.astype(np.float64) \
        + HW * bvs.astype(np.float64)
    base["cs_vs1"] = _chunk2(cs_vs1.astype(np.float32))

    import ml_dtypes
    for name in ["wvs", "w1", "w2", "w3", "w1c"]:
        base[name] = base[name].astype(ml_dtypes.bfloat16)

    in_maps = []
    for k in range(NCORES):
        m = dict(base)
        m["query_t"] = np.ascontiguousarray(
            query[:, k * JS:(k + 1) * JS].transpose(0, 2, 1)
            .reshape(B, 2, 128, JS)).astype(np.float32)
        rmask = np.array([1.0 if 0 <= 8 * k - 1 + lr < H else 0.0
                          for lr in range(10)], np.float32)
        m["rowmask"] = np.ascontiguousarray(
            np.tile(np.repeat(rmask, W)[None, :], (32, 1))).astype(ml_dtypes.bfloat16)
        in_maps.append(m)
    return in_maps


def run(in_maps, trace=False):
    from concourse.bass_utils import run_bass_kernel_spmd
    nc = _get_nc()
    return run_bass_kernel_spmd(nc, in_maps, core_ids=list(range(NCORES)),
                                trace=trace)


def kernel(**inputs):
    inputs = {k: np.asarray(v, dtype=np.float32) for k, v in inputs.items()}
    in_maps = _prep_inputs(**inputs)
    res = run(in_maps, trace=False)
    return np.concatenate([res.results[k]["out"] for k in range(NCORES)], axis=2)


if __name__ == "__main__":
    print("building...")
    _get_nc()
    print("built ok")
